# revision 1
# baseline (speedup 1.0000x reference)
"""AttnUpBlock2D Trainium2 kernel.

Pipeline per sample: bilinear up2 (align_corners) -> conv3x3(256->128)+BN+lrelu
-> conv3x3+BN+lrelu -> conv3x3+BN+lrelu -> self-attention (C=128, N=4096)
-> +identity -> lrelu.

Sharding: 8 cores = 4 samples x 2 spatial halves (32 of 64 output rows each).
Convs computed locally with halo rows (host ships pre-gathered, zero-padded
upsample operands). The attention needs the full feature map, so the two
cores of a sample AllGather their conv outputs, then each computes attention
for its own 2048 query positions. Softmax uses a constant exp-shift (exact
softmax for any constant) so no row-max pass is needed; row sums come from an
ones-vector matmul accumulated alongside the output matmul.

All matmuls run in float32r (fast fp32 mode, ~1.2e-4 rounding), PSUM f32.
"""

import os
import numpy as np

import concourse.bass as bass
import concourse.bacc as bacc
import concourse.tile as tile
from concourse import mybir
from concourse.bass_utils import run_bass_kernel_spmd

f32 = mybir.dt.float32
f32r = mybir.dt.float32r
bf16 = mybir.dt.bfloat16

B, CIN, C, HIN, WIN = 4, 256, 128, 32, 32
H, W = 64, 64                  # upsampled
N = H * W                      # 4096 positions per sample
HH = 32                        # rows per core (half)
M = HH * W                     # 2048 own positions per core
D = C // 2                     # 64 qk dim
EPS = 1e-5
ALPHA = 0.2
SHIFT = 40.0                   # exp shift; observed logit max ~53.6, shift keeps exp args <= ~14

UPR = 38                       # upsample rows computed per core ([-3, 35) rel to base)
Y0R = 36                       # y0 rows ([-2, 34))
Y1R = 34                       # y1 rows ([-1, 33))
WB = W + 2                     # padded width
N_CORES = 8
GROUPS = [[0, 1], [2, 3], [4, 5], [6, 7]]

_PROGRAM = None
LAST_RUN = None                # BassKernelResults of the most recent kernel() call


def _row_groups(nrows, step=8):
    out = []
    r = 0
    while r < nrows:
        out.append((r, min(step, nrows - r)))
        r += step
    return out


def build_program(reps=1):
    """reps>1 repeats the whole compute body (same output) — used only for
    differential hardware timing; the graded path uses reps=1."""
    global _PROGRAM
    if _PROGRAM is not None and reps == 1:
        return _PROGRAM

    nc = bacc.Bacc("TRN2", target_bir_lowering=False, debug=False,
                   num_devices=N_CORES)

    def din(name, shape):
        return nc.dram_tensor(name, list(shape), f32, kind="ExternalInput").ap()

    XWR = 22                         # x-row window per core (zero-padded)
    xw_ap = din("xw", [128, 2, XWR, WIN])
    upm_ap = din("upm", [128, UPR])
    w1a_ap = din("w1a", [128, UPR])
    w1bo_ap = din("w1bo", [128, WIN - 1])
    w1be_ap = din("w1be", [128, WIN - 1])
    wu_ap = din("wu", [128, 2, 9, C])
    w0_ap = din("w0", [128, 9, C])
    w1_ap = din("w1", [128, 9, C])
    b0_ap = din("b0", [C, 1])
    b1_ap = din("b1", [C, 1])
    b2_ap = din("b2", [C, 1])
    m0_ap = din("my0", [128, Y0R])
    m1_ap = din("my1", [128, Y1R])
    wq2_ap = din("wq2", [C, 2 * D])      # [Wq.T | Wq.T]
    bq2_ap = din("bq2", [128, 1])
    wk2_ap = din("wk2", [C, 2 * D])
    bk2_ap = din("bk2", [128, 1])
    wvt_ap = din("wvt", [C, C])
    bv_ap = din("bv", [C, 1])
    onesc_ap = din("onesc", [128, 1])
    out_ap = nc.dram_tensor("out", [C, HH, W], f32, kind="ExternalOutput").ap()

    # pair exchange runs in bf16: halves the wire bytes; the partner half
    # is recovered as (bf16 sum - own), within bf16 noise of the true remote
    y2_dram = nc.dram_tensor("y2d", [C, M], bf16).ap()
    ar_out = nc.dram_tensor("arout", [C, M], bf16).ap()

    with tile.TileContext(nc) as tc:
        from contextlib import ExitStack
        for _rep in range(reps):
          with ExitStack() as ctx:
              wp = ctx.enter_context(tc.tile_pool(name=f"wp{_rep}", bufs=1))
              sb = ctx.enter_context(tc.tile_pool(name=f"sb{_rep}", bufs=1))
              pts = ctx.enter_context(tc.tile_pool(name=f"pts{_rep}", bufs=3))
              # "st" slots ([128, 2, 512] = 2 banks x 2 bufs) serve the conv
              # groups, q/k/vT scratch AND the attention S^T pair tiles
              psS = ctx.enter_context(tc.tile_pool(name=f"psS{_rep}", bufs=2, space="PSUM"))
              psO = ctx.enter_context(tc.tile_pool(name=f"psO{_rep}", bufs=2, space="PSUM"))
              psA = ctx.enter_context(tc.tile_pool(name=f"psA{_rep}", bufs=2, space="PSUM"))

              # ---- input window first (upsample is the pipeline head) ----
              UP_BLOCKS = ((0, 10), (10, 20), (20, 30), (30, UPR))
              up = sb.tile([128, 2, UPR, WB], f32r)
              upp_cm = tc.tile_pool(name=f"upp{_rep}", bufs=1)
              upp = upp_cm.__enter__()
              xw = upp.tile([128, 2, XWR, WIN], f32r)
              nc.sync.dma_start(out=xw, in_=xw_ap.bitcast(f32r))
              if _rep > 0:
                  # serialize reps for differential timing: perturb xw by
                  # 0 * (previous rep's last output block) so rep N+1 can't
                  # start before rep N's final DMA lands
                  dummy = upp.tile([C, 2, WIN], f32)
                  nc.sync.dma_start(
                      out=dummy,
                      in_=out_ap[:, HH - 1:HH, :].rearrange(
                          "c r (a w) -> c (r a) w", a=2))
                  nc.vector.tensor_scalar_mul(out=dummy, in0=dummy, scalar1=0.0)
                  nc.vector.tensor_tensor(out=xw[:, :, 0, :],
                                          in0=xw[:, :, 0, :].bitcast(f32),
                                          in1=dummy,
                                          op=mybir.AluOpType.add)
              upm = wp.tile([128, UPR], f32, tag="upm")
              nc.sync.dma_start(out=upm, in_=upm_ap)

              # ---- constants / weights ----
              w1a = wp.tile([128, UPR], f32)
              nc.sync.dma_start(out=w1a, in_=w1a_ap)
              w1bo = wp.tile([128, WIN - 1], f32)
              nc.sync.dma_start(out=w1bo, in_=w1bo_ap)
              w1be = wp.tile([128, WIN - 1], f32)
              nc.sync.dma_start(out=w1be, in_=w1be_ap)
              wu = wp.tile([128, 2, 9, C], f32r)
              nc.sync.dma_start(out=wu, in_=wu_ap.bitcast(f32r))
              w0 = wp.tile([128, 9, C], f32r)
              nc.sync.dma_start(out=w0, in_=w0_ap.bitcast(f32r))
              w1t = wp.tile([128, 9, C], f32r)
              nc.sync.dma_start(out=w1t, in_=w1_ap.bitcast(f32r))
              b0 = wp.tile([C, 1], f32)
              nc.sync.dma_start(out=b0, in_=b0_ap)
              b1 = wp.tile([C, 1], f32)
              nc.sync.dma_start(out=b1, in_=b1_ap)
              b2 = wp.tile([C, 1], f32)
              nc.sync.dma_start(out=b2, in_=b2_ap)
              my0 = wp.tile([128, Y0R], f32)
              nc.sync.dma_start(out=my0, in_=m0_ap)
              my1 = wp.tile([128, Y1R], f32)
              nc.sync.dma_start(out=my1, in_=m1_ap)
              # q/k projection weights duplicated side by side: one matmul
              # writes q (partitions 0..63) and its copy (64..127), which the
              # paired S^T matmuls read directly — no dup copy needed.
              wq2 = wp.tile([C, 2 * D], f32r)
              nc.sync.dma_start(out=wq2, in_=wq2_ap.bitcast(f32r))
              wk2 = wp.tile([C, 2 * D], f32r)
              nc.sync.dma_start(out=wk2, in_=wk2_ap.bitcast(f32r))
              # wvt duplicated side by side: the vT matmuls then have a
              # 256-wide moving operand (1 cyc/row in f32r vs 4 at 128)
              wvt2 = wp.tile([C, 2, C], f32r)
              for _cp in range(2):
                  nc.sync.dma_start(out=wvt2[:, _cp, :], in_=wvt_ap.bitcast(f32r))
              bq2 = wp.tile([128, 1], f32)
              nc.sync.dma_start(out=bq2, in_=bq2_ap)
              bk2 = wp.tile([128, 1], f32)
              nc.sync.dma_start(out=bk2, in_=bk2_ap)
              bv = wp.tile([C, 1], f32)
              nc.sync.dma_start(out=bv, in_=bv_ap)
              onesc = wp.tile([128, 1], f32r)
              nc.sync.dma_start(out=onesc, in_=onesc_ap.bitcast(f32r))
              alpha = wp.tile([C, 1], f32)
              nc.vector.memset(alpha, ALPHA)
              nshift = wp.tile([128, 1], f32)
              nc.vector.memset(nshift, -SHIFT)

              # ---- upsample ----
              # Row interp: output rows t=2a and 2a+1 both read x-window
              # rows a, a+1 (align-corners grid: i0(j) = (j-1)//2, and the
              # row phase is identical for both halves since base is
              # even); per-t weights w1a; out-of-image rows zeroed by upm.
              dr = upp.tile([128, 2, XWR - 1, WIN], f32r)
              nc.vector.tensor_tensor(out=dr,
                                      in0=xw[:, :, 1:, :].bitcast(f32),
                                      in1=xw[:, :, :-1, :].bitcast(f32),
                                      op=mybir.AluOpType.subtract)
              xr = upp.tile([128, 2, UPR, WIN], f32r)
              dc = upp.tile([128, 2, UPR, WIN - 1], f32r)
              nc.vector.memset(up[:, :, :, 0:1].bitcast(f32), 0.0)
              nc.vector.memset(up[:, :, :, WB - 1:WB].bitcast(f32), 0.0)
              for (r0b, r1b) in UP_BLOCKS:
                  nr = r1b - r0b
                  a0 = r0b // 2
                  na = nr // 2 + (nr % 2)
                  xrk = xr[:, :, r0b:r1b, :]
                  # DVE APs allow at most 3 free dims, so the paired-row
                  # gather runs per channel-chunk
                  w1a_b = bass.AP(tensor=w1a.tensor,
                                  offset=w1a.offset + r0b,
                                  ap=[w1a.ap[0], [1, nr], [0, WIN]])
                  upm_b = bass.AP(tensor=upm.tensor,
                                  offset=upm.offset + r0b,
                                  ap=[upm.ap[0], [1, nr], [0, WIN]])
                  for ch in range(2):
                      def _pairs(t, row_stride, chunk_stride):
                          return bass.AP(
                              tensor=t.tensor,
                              offset=t.offset + ch * chunk_stride
                              + a0 * row_stride,
                              ap=[t.ap[0], [row_stride, na], [0, 2],
                                  [1, WIN]])
                      drv = _pairs(dr.bitcast(f32), WIN, (XWR - 1) * WIN)
                      xwv = _pairs(xw.bitcast(f32), WIN, XWR * WIN)
                      xrc = xrk[:, ch, :, :]
                      nc.vector.tensor_tensor(out=xrc, in0=drv, in1=w1a_b,
                                              op=mybir.AluOpType.mult)
                      nc.vector.tensor_tensor(out=xrc, in0=xrc.bitcast(f32),
                                              in1=xwv, op=mybir.AluOpType.add)
                      nc.vector.tensor_tensor(out=xrc, in0=xrc.bitcast(f32),
                                              in1=upm_b,
                                              op=mybir.AluOpType.mult)
                  dck = dc[:, :, r0b:r1b, :]
                  nc.vector.tensor_tensor(out=dck,
                                          in0=xrk[:, :, :, 1:].bitcast(f32),
                                          in1=xrk[:, :, :, :-1].bitcast(f32),
                                          op=mybir.AluOpType.subtract)
                  nc.vector.tensor_copy(up[:, :, r0b:r1b, 1],
                                        xrk[:, :, :, 0].bitcast(f32))
                  nc.vector.tensor_copy(up[:, :, r0b:r1b, 1 + (W - 1)],
                                        xrk[:, :, :, WIN - 1].bitcast(f32))
                  dc_f = dck.bitcast(f32)[:, :, :, 0:WIN - 1]
                  xr_f = xrk.bitcast(f32)[:, :, :, 0:WIN - 1]
                  for (wt, col0) in ((w1bo, 2), (w1be, 3)):
                      out_v = bass.AP(tensor=up.tensor,
                                      offset=up.offset + r0b * WB + col0,
                                      ap=[up.ap[0], up.ap[1], [WB, nr],
                                          [2, WIN - 1]])
                      wt_b = bass.AP(tensor=wt.tensor, offset=wt.offset,
                                     ap=[wt.ap[0], [0, 2], [0, nr],
                                         wt.ap[1]])
                      nc.vector.tensor_tensor(out=out_v, in0=dc_f,
                                              in1=wt_b,
                                              op=mybir.AluOpType.mult)
                      nc.vector.tensor_tensor(out=out_v,
                                              in0=out_v.bitcast(f32),
                                              in1=xr_f,
                                              op=mybir.AluOpType.add)

              upp_cm.__exit__(None, None, None)

              # ---- conv_up (256->128) + bn0 + lrelu -> y0 ----
              y0 = sb.tile([C, Y0R, WB], f32r)
              nc.vector.memset(y0[:, :, 0:1].bitcast(f32), 0.0)
              nc.vector.memset(y0[:, :, WB - 1:WB].bitcast(f32), 0.0)
              for (u0, nr) in _row_groups(Y0R):
                  pt = psS.tile([C, nr * W], f32, tag="st")
                  first = True
                  for dy in range(3):
                      for dx in range(3):
                          k = 3 * dy + dx
                          for ch in range(2):
                              nc.tensor.matmul(
                                  pt, wu[:, ch, k, :],
                                  up[:, ch, u0 + dy:u0 + dy + nr, dx:dx + W],
                                  start=first, stop=(k == 8 and ch == 1))
                              first = False
                  nc.scalar.activation(out=y0[:, u0:u0 + nr, 1:1 + W],
                                       in_=pt.rearrange("p (r w) -> p r w", r=nr),
                                       func=mybir.ActivationFunctionType.Prelu,
                                       bias=b0, scale=1.0, alpha=alpha)
              # mask out-of-image halo rows (data-driven per core); only the
              # first/last two rows can ever be masked, keep the ops tiny
              for rr in (0, Y0R - 2):
                  nc.vector.tensor_tensor(
                      out=y0[:, rr:rr + 2, 1:1 + W],
                      in0=y0[:, rr:rr + 2, 1:1 + W].bitcast(f32),
                      in1=bass.AP(tensor=my0.tensor, offset=my0.offset + rr,
                                  ap=[my0.ap[0], [1, 2], [0, W]]),
                      op=mybir.AluOpType.mult)

              # ---- conv r0 + bn1 + lrelu -> y1 ----
              y1 = sb.tile([C, Y1R, WB], f32r)
              nc.vector.memset(y1[:, :, 0:1].bitcast(f32), 0.0)
              nc.vector.memset(y1[:, :, WB - 1:WB].bitcast(f32), 0.0)
              for (v0, nr) in _row_groups(Y1R):
                  pt = psS.tile([C, nr * W], f32, tag="st")
                  for dy in range(3):
                      for dx in range(3):
                          k = 3 * dy + dx
                          nc.tensor.matmul(
                              pt, w0[:, k, :],
                              y0[:, v0 + dy:v0 + dy + nr, dx:dx + W],
                              start=(k == 0), stop=(k == 8))
                  nc.scalar.activation(out=y1[:, v0:v0 + nr, 1:1 + W],
                                       in_=pt.rearrange("p (r w) -> p r w", r=nr),
                                       func=mybir.ActivationFunctionType.Prelu,
                                       bias=b1, scale=1.0, alpha=alpha)
              for rr in (0, Y1R - 1):
                  nc.vector.tensor_tensor(
                      out=y1[:, rr:rr + 1, 1:1 + W],
                      in0=y1[:, rr:rr + 1, 1:1 + W].bitcast(f32),
                      in1=bass.AP(tensor=my1.tensor, offset=my1.offset + rr,
                                  ap=[my1.ap[0], [1, 1], [0, W]]),
                      op=mybir.AluOpType.mult)

              # ---- conv r1 + bn2 + lrelu -> y2 (flat [C, M]) ----
              y2 = sb.tile([C, HH, W], f32r)
              y2h = sb.tile([C, HH, W], bf16)
              for (z0, nr) in _row_groups(HH):
                  pt = psS.tile([C, nr * W], f32, tag="st")
                  for dy in range(3):
                      for dx in range(3):
                          k = 3 * dy + dx
                          nc.tensor.matmul(
                              pt, w1t[:, k, :],
                              y1[:, z0 + dy:z0 + dy + nr, dx:dx + W],
                              start=(k == 0), stop=(k == 8))
                  nc.scalar.activation(out=y2[:, z0:z0 + nr, :],
                                       in_=pt.rearrange("p (r w) -> p r w", r=nr),
                                       func=mybir.ActivationFunctionType.Prelu,
                                       bias=b2, scale=1.0, alpha=alpha)
                  nc.vector.tensor_copy(y2h[:, z0:z0 + nr, :],
                                        y2[:, z0:z0 + nr, :].bitcast(f32))
                  nc.sync.dma_start(
                      out=y2_dram[:, z0 * W:(z0 + nr) * W],
                      in_=y2h[:, z0:z0 + nr, :].rearrange("p r w -> p (r w)"))
              y2f = y2.rearrange("p r w -> p (r w)")

              # residual sum (attention adds xf + identity): y20 = y2 + y0_core
              # + bv (the attention v-bias collapses to a constant add since
              # softmax rows sum to 1), so the final lrelu is bias-free and
              # can run on the DVE instead of the busy scalar engine
              y20 = sb.tile([C, M], f32)
              nc.vector.tensor_tensor(
                  out=y20.rearrange("p (r w) -> p r w", r=HH),
                  in0=y2, in1=y0[:, 2:2 + HH, 1:1 + W].bitcast(f32),
                  op=mybir.AluOpType.add)
              bv_b = bass.AP(tensor=bv.tensor, offset=bv.offset,
                             ap=[bv.ap[0], [0, M]])
              nc.vector.tensor_tensor(out=y20, in0=y20, in1=bv_b,
                                      op=mybir.AluOpType.add)

              # ---- q (local only: overlaps the AllReduce) ----
              bq2_b = bass.AP(tensor=bq2.tensor, offset=bq2.offset,
                              ap=[bq2.ap[0], [0, 512]])
              # q2: [wq|wq] stationary writes q into partitions 0..63 AND a
              # copy into 64..127 in one matmul, so S^T n-chunk pairs can run
              # as concurrent row-group-packed matmuls.
              q2 = sb.tile([128, M], f32r)
              for c0 in range(0, M, 512):
                  pq = psS.tile([128, 512], f32, tag="st")
                  nc.tensor.matmul(pq, wq2, y2f[:, c0:c0 + 512], start=True, stop=True)
                  nc.vector.tensor_tensor(out=q2[:, c0:c0 + 512], in0=pq,
                                          in1=bq2_b, op=mybir.AluOpType.add)

              # ---- pair exchange: one AllReduce(add) per conv group pair;
              # the partner half is then (sum - own). Each core's attention
              # uses its own n-ordering [local | partner] (order-invariant).
              if os.environ.get("KERNEL_NO_COLLECTIVE", "0") == "1":
                  # timing probe only (wrong results): local copy in place of
                  # the pair exchange, to measure the collective's HW cost
                  nc.sync.dma_start(out=ar_out, in_=y2_dram)
              else:
                  nc.gpsimd.collective_compute(
                      "AllReduce", mybir.AluOpType.add,
                      replica_groups=GROUPS,
                      ins=[y2_dram.opt()],
                      outs=[ar_out.opt()])
              xremh = sb.tile([C, M], bf16)
              xrem = sb.tile([C, M], f32r)
              for c0 in range(0, M, 512):
                  nc.sync.dma_start(out=xremh[:, c0:c0 + 512],
                                    in_=ar_out[:, c0:c0 + 512])
                  nc.vector.tensor_tensor(
                      out=xrem[:, c0:c0 + 512],
                      in0=xremh[:, c0:c0 + 512],
                      in1=y2f[:, c0:c0 + 512].bitcast(f32),
                      op=mybir.AluOpType.subtract)

              # ---- k, vT ----
              bk2_b = bass.AP(tensor=bk2.tensor, offset=bk2.offset,
                              ap=[bk2.ap[0], [0, 512]])
              # local and remote halves live in SEPARATE tiles and are
              # emitted in a schedule that keeps ~30us of local-only work
              # queued ahead of the first collective-dependent instruction:
              # engines drain queues in order, so anything emitted after a
              # stalled instruction waits with it.
              MJ = M // 128
              k2h = [sb.tile([128, M], f32r, name=f"k2h{h}")
                     for h in range(2)]
              vTh = [sb.tile([128, MJ, C], f32r, name=f"vTh{h}")
                     for h in range(2)]

              def project_half(half):
                  src_half = y2f if half == 0 else xrem
                  for c0 in range(0, M, 512):
                      pk = psS.tile([128, 512], f32, tag="st")
                      nc.tensor.matmul(pk, wk2, src_half[:, c0:c0 + 512],
                                       start=True, stop=True)
                      nc.vector.tensor_tensor(out=k2h[half][:, c0:c0 + 512],
                                              in0=pk, in1=bk2_b,
                                              op=mybir.AluOpType.add)
                  for jj in range(MJ):
                      pv = psS.tile([128, 2, C], f32, tag="st")
                      nc.tensor.matmul(pv, src_half[:, jj * 128:(jj + 1) * 128],
                                       wvt2.rearrange("p a b -> p (a b)"),
                                       start=True, stop=True)
                      if half == 0 and jj % 2 == 1:
                          nc.scalar.activation(
                              out=vTh[half][:, jj, :], in_=pv[:, 0, :],
                              func=mybir.ActivationFunctionType.Copy)
                      else:
                          nc.vector.tensor_copy(vTh[half][:, jj, :],
                                                pv[:, 0, :])

              NJ = N // 128

              def attn_quarter(ms, pO, psums, half):
                  mlo = ms * 512
                  for j0 in range(half * NJ // 2, half * NJ // 2 + NJ // 2, 2):
                      pS = psS.tile([128, 2, 512], f32, tag="st")
                      for u in range(2):
                          j = j0 + u
                          bp0 = u * D
                          nc.tensor.matmul(pS[:, u, :],
                                           k2h[j // MJ][bp0:bp0 + D,
                                                        (j % MJ) * 128:
                                                        (j % MJ + 1) * 128],
                                           q2[bp0:bp0 + D, mlo:mlo + 512],
                                           start=True, stop=True,
                                           tile_position=(bp0, 0))
                      pt = pts.tile([128, 2, 512], f32r, tag="pt")
                      nc.scalar.activation(out=pt, in_=pS,
                                           func=mybir.ActivationFunctionType.Exp,
                                           bias=nshift, scale=1.0)
                      for u in range(2):
                          j = j0 + u
                          nc.tensor.matmul(pO, vTh[j // MJ][:, j % MJ, :],
                                           pt[:, u, :],
                                           start=(j == 0), stop=(j == NJ - 1))
                          nc.tensor.matmul(psums, onesc, pt[:, u, :],
                                           start=(j == 0), stop=(j == NJ - 1))

              def attn_finish(ms, pO, psums):
                  mlo = ms * 512
                  recip = pts.tile([1, 512], f32, tag="rc")
                  with nc.allow_low_precision(reason="softmax denominator"):
                      nc.vector.reciprocal(out=recip, in_=psums)
                  rbs = pts.tile([128, 512], f32, tag="rb")
                  nc.gpsimd.partition_broadcast(rbs, recip)
                  onorm = pts.tile([C, 512], f32, tag="on")
                  nc.vector.tensor_tensor(out=onorm, in0=pO, in1=rbs,
                                          op=mybir.AluOpType.mult)
                  nc.vector.tensor_tensor(out=onorm, in0=onorm,
                                          in1=y20[:, mlo:mlo + 512],
                                          op=mybir.AluOpType.add)
                  osb = pts.tile([C, 512], f32, tag="ob")
                  nc.vector.tensor_scalar_mul(out=osb, in0=onorm, scalar1=ALPHA)
                  nc.vector.tensor_tensor(out=osb, in0=osb, in1=onorm,
                                          op=mybir.AluOpType.max)
                  nc.sync.dma_start(out=out_ap[:, ms * 8:(ms + 1) * 8, :],
                                    in_=osb.rearrange("p (r w) -> p r w", r=8))

              project_half(0)
              # ms0/ms1 local-half attention: queued ahead of anything that
              # needs the collective
              pO0 = psO.tile([C, 512], f32, tag="po")
              ps0 = psA.tile([1, 512], f32, tag="psum")
              pO1 = psO.tile([C, 512], f32, tag="po")
              ps1 = psA.tile([1, 512], f32, tag="psum")
              attn_quarter(0, pO0, ps0, 0)
              attn_quarter(1, pO1, ps1, 0)
              # remote-half projections (first collective-dependent PE work)
              project_half(1)
              attn_quarter(0, pO0, ps0, 1)
              attn_finish(0, pO0, ps0)
              attn_quarter(1, pO1, ps1, 1)
              attn_finish(1, pO1, ps1)
              for ms in (2, 3):
                  pO = psO.tile([C, 512], f32, tag="po")
                  psums = psA.tile([1, 512], f32, tag="psum")
                  attn_quarter(ms, pO, psums, 0)
                  attn_quarter(ms, pO, psums, 1)
                  attn_finish(ms, pO, psums)

    nc.compile()
    if reps == 1:
        _PROGRAM = nc
    return nc


def _prep_inputs(x, W_up, b_up, g0, be0, m0, v0, W_r0, g1, be1, m1, v1,
                 W_r1, g2, be2, m2, v2, Wq, bq, Wk, bk, Wv, bv):
    """Build the 8 per-core input maps (host-side sharding/packing only)."""
    x = np.asarray(x, np.float32)

    def fold(wc, scale):
        return (wc * scale[:, None, None, None]).astype(np.float32)

    def pack(wc):  # [co, ci, 3, 3] -> [ci, 9, co]
        return np.ascontiguousarray(
            wc.transpose(1, 2, 3, 0).reshape(wc.shape[1], 9, wc.shape[0]))

    s0 = np.asarray(g0) / np.sqrt(np.asarray(v0) + EPS)
    s1 = np.asarray(g1) / np.sqrt(np.asarray(v1) + EPS)
    s2 = np.asarray(g2) / np.sqrt(np.asarray(v2) + EPS)
    b0f = (np.asarray(b_up) * s0 + np.asarray(be0) - np.asarray(m0) * s0)
    b1f = (np.asarray(be1) - np.asarray(m1) * s1)
    b2f = (np.asarray(be2) - np.asarray(m2) * s2)

    wu_p = pack(fold(np.asarray(W_up), s0))      # [256, 9, 128]
    wu_p = wu_p.reshape(2, 128, 9, C).transpose(1, 0, 2, 3)
    wu_p = np.ascontiguousarray(wu_p, np.float32)
    w0_p = np.ascontiguousarray(pack(fold(np.asarray(W_r0), s1)), np.float32)
    w1_p = np.ascontiguousarray(pack(fold(np.asarray(W_r1), s2)), np.float32)

    co = np.linspace(0.0, HIN - 1.0, H)
    i0 = np.floor(co).astype(np.int64)
    i1 = np.minimum(i0 + 1, HIN - 1)
    wrow = (co - i0).astype(np.float32)
    w1b_col = (co - i0).astype(np.float32)       # same grid for W axis
    w1bo_t = np.broadcast_to(w1b_col[1:63:2][None, :], (128, WIN - 1)).copy()
    w1be_t = np.broadcast_to(w1b_col[2:63:2][None, :], (128, WIN - 1)).copy()

    wqt = np.ascontiguousarray(np.asarray(Wq).T, np.float32)
    wkt = np.ascontiguousarray(np.asarray(Wk).T, np.float32)
    wq2 = np.ascontiguousarray(np.concatenate([wqt, wqt], axis=1), np.float32)
    wk2 = np.ascontiguousarray(np.concatenate([wkt, wkt], axis=1), np.float32)
    wvt = np.ascontiguousarray(np.asarray(Wv).T, np.float32)
    bq2_c = np.concatenate([np.asarray(bq), np.asarray(bq)]).astype(
        np.float32).reshape(2 * D, 1)
    bk2_c = np.concatenate([np.asarray(bk), np.asarray(bk)]).astype(
        np.float32).reshape(2 * D, 1)
    bv_c = np.asarray(bv, np.float32).reshape(C, 1)
    b0c = b0f.astype(np.float32).reshape(C, 1)
    b1c = b1f.astype(np.float32).reshape(C, 1)
    b2c = b2f.astype(np.float32).reshape(C, 1)

    XWR = 22
    in_maps = []
    for core in range(N_CORES):
        s, h = core // 2, core % 2
        base = HH * h
        xs = x[s]                                # [256, 32, 32]
        rlo = base // 2 - 2
        xw = np.zeros((CIN, XWR, WIN), np.float32)
        for r in range(XWR):
            xr_idx = rlo + r
            if 0 <= xr_idx < HIN:
                xw[:, r, :] = xs[:, xr_idx, :]
        xw = np.ascontiguousarray(
            xw.reshape(2, 128, XWR, WIN).transpose(1, 0, 2, 3))
        w1a = np.zeros((UPR,), np.float32)
        upm = np.zeros((UPR,), np.float32)
        for t in range(UPR):
            j = base - 3 + t
            if 0 <= j < H:
                upm[t] = 1.0
                # j==0 is the exact-sample row: through the pair formula
                # x[rel a] + w*(x[rel a+1]-x[rel a]) with w=1 it returns
                # x[rel a+1] = x row 0 exactly
                w1a[t] = 1.0 if j == 0 else wrow[j]
        w1a_t = np.broadcast_to(w1a[None, :], (128, UPR)).copy()
        upm_t = np.broadcast_to(upm[None, :], (128, UPR)).copy()

        my0 = np.ones((Y0R,), np.float32)
        my1 = np.ones((Y1R,), np.float32)
        if h == 0:
            my0[0:2] = 0.0                       # y0 rows -2,-1
            my1[0] = 0.0                         # y1 row -1
        else:
            my0[Y0R - 2:] = 0.0                  # y0 rows 64,65
            my1[Y1R - 1] = 0.0                   # y1 row 64
        in_maps.append(dict(
            xw=xw, upm=upm_t,
            w1a=w1a_t, w1bo=w1bo_t, w1be=w1be_t,
            wu=wu_p, w0=w0_p, w1=w1_p,
            b0=b0c, b1=b1c, b2=b2c,
            my0=np.broadcast_to(my0[None, :], (128, Y0R)).copy(),
            my1=np.broadcast_to(my1[None, :], (128, Y1R)).copy(),
            wq2=wq2, bq2=bq2_c, wk2=wk2, bk2=bk2_c, wvt=wvt, bv=bv_c,
            onesc=np.ones((128, 1), np.float32),
        ))
    return in_maps


def kernel(**inputs):
    global LAST_RUN
    nc = build_program()
    in_maps = _prep_inputs(**inputs)
    trace = bool(int(os.environ.get("KERNEL_TRACE", "0")))
    res = run_bass_kernel_spmd(nc, in_maps, list(range(N_CORES)), trace=trace)
    LAST_RUN = res
    out = np.empty((B, C, H, W), np.float32)
    for core in range(N_CORES):
        s, h = core // 2, core % 2
        out[s, :, HH * h:HH * (h + 1), :] = res.results[core]["out"]
    return out



# revision 10
# speedup vs baseline: 1.2443x; 1.2443x over previous
"""AttnUpBlock2D Trainium2 kernel.

Pipeline per sample: bilinear up2 (align_corners) -> conv3x3(256->128)+BN+lrelu
-> conv3x3+BN+lrelu -> conv3x3+BN+lrelu -> self-attention (C=128, N=4096)
-> +identity -> lrelu.

Sharding: 8 cores = 4 samples x 2 spatial halves (32 of 64 output rows each).
Convs computed locally with halo rows (host ships pre-gathered, zero-padded
upsample operands). The attention needs the full feature map, so the two
cores of a sample AllGather their conv outputs, then each computes attention
for its own 2048 query positions. Softmax uses a constant exp-shift (exact
softmax for any constant) so no row-max pass is needed; row sums come from an
ones-vector matmul accumulated alongside the output matmul.

All matmuls run in float32r (fast fp32 mode, ~1.2e-4 rounding), PSUM f32.
"""

import os
import numpy as np

import concourse.bass as bass
import concourse.bacc as bacc
import concourse.tile as tile
from concourse import mybir
from concourse.bass_utils import run_bass_kernel_spmd

f32 = mybir.dt.float32
f32r = mybir.dt.float32r
bf16 = mybir.dt.bfloat16

B, CIN, C, HIN, WIN = 4, 256, 128, 32, 32
H, W = 64, 64                  # upsampled
N = H * W                      # 4096 positions per sample
HH = 32                        # rows per core (half)
M = HH * W                     # 2048 own positions per core
D = C // 2                     # 64 qk dim
EPS = 1e-5
ALPHA = 0.2
SHIFT = 40.0                   # exp shift; observed logit max ~53.6, shift keeps exp args <= ~14

UPR = 38                       # upsample rows computed per core ([-3, 35) rel to base)
Y0R = 36                       # y0 rows ([-2, 34))
Y1R = 34                       # y1 rows ([-1, 33))
WB = W + 2                     # padded width
N_CORES = 8
GROUPS = [[0, 1], [2, 3], [4, 5], [6, 7]]

_PROGRAM = None
LAST_RUN = None                # BassKernelResults of the most recent kernel() call


def _row_groups(nrows, step=8):
    out = []
    r = 0
    while r < nrows:
        out.append((r, min(step, nrows - r)))
        r += step
    return out


def build_program(reps=1):
    """reps>1 repeats the whole compute body (same output) — used only for
    differential hardware timing; the graded path uses reps=1."""
    global _PROGRAM
    if _PROGRAM is not None and reps == 1:
        return _PROGRAM

    nc = bacc.Bacc("TRN2", target_bir_lowering=False, debug=False,
                   num_devices=N_CORES)

    def din(name, shape):
        return nc.dram_tensor(name, list(shape), f32, kind="ExternalInput").ap()

    XWR = 22                         # x-row window per core (zero-padded)
    xw_ap = din("xw", [128, 2, XWR, WIN])
    upm_ap = din("upm", [128, UPR])
    w1a_ap = din("w1a", [128, UPR])
    w1bo_ap = din("w1bo", [128, WIN - 1])
    w1be_ap = din("w1be", [128, WIN - 1])
    wu_ap = din("wu", [128, 2, 9, C])
    w0_ap = din("w0", [128, 9, C])
    w1_ap = din("w1", [128, 9, C])
    b0_ap = din("b0", [C, 1])
    b1_ap = din("b1", [C, 1])
    b2_ap = din("b2", [C, 1])
    m0_ap = din("my0", [128, Y0R])
    m1_ap = din("my1", [128, Y1R])
    wq2_ap = din("wq2", [C, 2 * D])      # [Wq.T | Wq.T]
    bq2_ap = din("bq2", [128, 1])
    wk2_ap = din("wk2", [C, 2 * D])
    bk2_ap = din("bk2", [128, 1])
    wvt_ap = din("wvt", [C, C])
    bv_ap = din("bv", [C, 1])
    onesc_ap = din("onesc", [128, 1])
    out_ap = nc.dram_tensor("out", [C, HH, W], f32, kind="ExternalOutput").ap()

    # pair exchange runs in bf16: halves the wire bytes; the partner half
    # is recovered as (bf16 sum - own), within bf16 noise of the true remote
    y2_dram = nc.dram_tensor("y2d", [C, M], bf16).ap()
    ar_out = nc.dram_tensor("arout", [C, M], bf16).ap()

    with tile.TileContext(nc) as tc:
        from contextlib import ExitStack
        for _rep in range(reps):
          with ExitStack() as ctx:
              wp = ctx.enter_context(tc.tile_pool(name=f"wp{_rep}", bufs=1))
              sb = ctx.enter_context(tc.tile_pool(name=f"sb{_rep}", bufs=1))
              pts = ctx.enter_context(tc.tile_pool(name=f"pts{_rep}", bufs=4))
              # "st" slots ([128, 512] = 1 bank x 4 bufs) serve the conv
              # groups, q/k/vT scratch AND the attention S^T tiles
              psS = ctx.enter_context(tc.tile_pool(name=f"psS{_rep}", bufs=4, space="PSUM"))
              psO = ctx.enter_context(tc.tile_pool(name=f"psO{_rep}", bufs=2, space="PSUM"))
              psA = ctx.enter_context(tc.tile_pool(name=f"psA{_rep}", bufs=2, space="PSUM"))

              # ---- input window first (upsample is the pipeline head) ----
              # conv-phase-only tensors live in their own pool, exited before
              # the attention tiles allocate, so SBUF is reused
              convp_cm = tc.tile_pool(name=f"convp{_rep}", bufs=1)
              convp = convp_cm.__enter__()
              UP_BLOCKS = ((0, 10), (10, 20), (20, 30), (30, UPR))
              up = convp.tile([128, 2, UPR, WB], f32r)
              upp_cm = tc.tile_pool(name=f"upp{_rep}", bufs=1)
              upp = upp_cm.__enter__()
              xw = upp.tile([128, 2, XWR, WIN], f32r)
              nc.sync.dma_start(out=xw, in_=xw_ap.bitcast(f32r))
              if _rep > 0:
                  # serialize reps for differential timing: perturb xw by
                  # 0 * (previous rep's last output block) so rep N+1 can't
                  # start before rep N's final DMA lands
                  dummy = upp.tile([C, 2, WIN], f32)
                  nc.sync.dma_start(
                      out=dummy,
                      in_=out_ap[:, HH - 1:HH, :].rearrange(
                          "c r (a w) -> c (r a) w", a=2))
                  nc.vector.tensor_scalar_mul(out=dummy, in0=dummy, scalar1=0.0)
                  nc.vector.tensor_tensor(out=xw[:, :, 0, :],
                                          in0=xw[:, :, 0, :].bitcast(f32),
                                          in1=dummy,
                                          op=mybir.AluOpType.add)
              upm = wp.tile([128, UPR], f32, tag="upm")
              nc.sync.dma_start(out=upm, in_=upm_ap)

              # ---- constants / weights ----
              w1a = wp.tile([128, UPR], f32)
              nc.sync.dma_start(out=w1a, in_=w1a_ap)
              w1bo = wp.tile([128, WIN - 1], f32)
              nc.sync.dma_start(out=w1bo, in_=w1bo_ap)
              w1be = wp.tile([128, WIN - 1], f32)
              nc.sync.dma_start(out=w1be, in_=w1be_ap)
              wu = wp.tile([128, 2, 9, C], f32r)
              nc.sync.dma_start(out=wu, in_=wu_ap.bitcast(f32r))
              w0 = wp.tile([128, 9, C], f32r)
              nc.sync.dma_start(out=w0, in_=w0_ap.bitcast(f32r))
              w1t = wp.tile([128, 9, C], f32r)
              nc.sync.dma_start(out=w1t, in_=w1_ap.bitcast(f32r))
              b0 = wp.tile([C, 1], f32)
              nc.sync.dma_start(out=b0, in_=b0_ap)
              b1 = wp.tile([C, 1], f32)
              nc.sync.dma_start(out=b1, in_=b1_ap)
              b2 = wp.tile([C, 1], f32)
              nc.sync.dma_start(out=b2, in_=b2_ap)
              my0 = wp.tile([128, Y0R], f32)
              nc.sync.dma_start(out=my0, in_=m0_ap)
              my1 = wp.tile([128, Y1R], f32)
              nc.sync.dma_start(out=my1, in_=m1_ap)
              # q/k projection weights duplicated side by side: one matmul
              # writes q (partitions 0..63) and its copy (64..127), which the
              # paired S^T matmuls read directly — no dup copy needed.
              wq2 = wp.tile([C, 2 * D], f32r)
              nc.sync.dma_start(out=wq2, in_=wq2_ap.bitcast(f32r))
              wk2 = wp.tile([C, 2 * D], f32r)
              nc.sync.dma_start(out=wk2, in_=wk2_ap.bitcast(f32r))
              # wvt duplicated side by side: the vT matmuls then have a
              # 256-wide moving operand (1 cyc/row in f32r vs 4 at 128)
              wvt2 = wp.tile([C, 2, C], f32r)
              for _cp in range(2):
                  nc.sync.dma_start(out=wvt2[:, _cp, :], in_=wvt_ap.bitcast(f32r))
              bq2 = wp.tile([128, 1], f32)
              nc.sync.dma_start(out=bq2, in_=bq2_ap)
              bk2 = wp.tile([128, 1], f32)
              nc.sync.dma_start(out=bk2, in_=bk2_ap)
              bv = wp.tile([C, 1], f32)
              nc.sync.dma_start(out=bv, in_=bv_ap)
              onesc = wp.tile([128, 1], f32r)
              nc.sync.dma_start(out=onesc, in_=onesc_ap.bitcast(f32r))
              alpha = wp.tile([C, 1], f32)
              nc.vector.memset(alpha, ALPHA)
              nshift = wp.tile([128, 1], f32)
              nc.vector.memset(nshift, -SHIFT)

              # ---- conv_up emission helper (interleaved with upsample
              # blocks below so the PE starts as soon as the first 10
              # upsampled rows land instead of after the full window) ----
              y0 = sb.tile([C, Y0R, WB], f32r)
              nc.vector.memset(y0[:, :, 0:1].bitcast(f32), 0.0)
              nc.vector.memset(y0[:, :, WB - 1:WB].bitcast(f32), 0.0)

              def emit_conv_up_group(u0, nr):
                  pt = psS.tile([C, nr * W], f32, tag="st")
                  first = True
                  for dy in range(3):
                      for dx in range(3):
                          k = 3 * dy + dx
                          for ch in range(2):
                              nc.tensor.matmul(
                                  pt, wu[:, ch, k, :],
                                  up[:, ch, u0 + dy:u0 + dy + nr, dx:dx + W],
                                  start=first, stop=(k == 8 and ch == 1))
                              first = False
                  nc.scalar.activation(out=y0[:, u0:u0 + nr, 1:1 + W],
                                       in_=pt.rearrange("p (r w) -> p r w", r=nr),
                                       func=mybir.ActivationFunctionType.Prelu,
                                       bias=b0, scale=1.0, alpha=alpha)

              cg_list = _row_groups(Y0R)
              cg_next = 0

              # ---- upsample ----
              # Row interp: output rows t=2a and 2a+1 both read x-window
              # rows a, a+1 (align-corners grid: i0(j) = (j-1)//2, and the
              # row phase is identical for both halves since base is
              # even); per-t weights w1a; out-of-image rows zeroed by upm.
              dr = upp.tile([128, 2, XWR - 1, WIN], f32r)
              nc.vector.tensor_tensor(out=dr,
                                      in0=xw[:, :, 1:, :].bitcast(f32),
                                      in1=xw[:, :, :-1, :].bitcast(f32),
                                      op=mybir.AluOpType.subtract)
              xr = upp.tile([128, 2, UPR, WIN], f32r)
              dc = upp.tile([128, 2, UPR, WIN - 1], f32r)
              nc.vector.memset(up[:, :, :, 0:1].bitcast(f32), 0.0)
              nc.vector.memset(up[:, :, :, WB - 1:WB].bitcast(f32), 0.0)
              for (r0b, r1b) in UP_BLOCKS:
                  nr = r1b - r0b
                  a0 = r0b // 2
                  na = nr // 2 + (nr % 2)
                  xrk = xr[:, :, r0b:r1b, :]
                  # DVE APs allow at most 3 free dims, so the paired-row
                  # gather runs per channel-chunk
                  w1a_b = bass.AP(tensor=w1a.tensor,
                                  offset=w1a.offset + r0b,
                                  ap=[w1a.ap[0], [1, nr], [0, WIN]])
                  upm_b = bass.AP(tensor=upm.tensor,
                                  offset=upm.offset + r0b,
                                  ap=[upm.ap[0], [1, nr], [0, WIN]])
                  for ch in range(2):
                      def _pairs(t, row_stride, chunk_stride):
                          return bass.AP(
                              tensor=t.tensor,
                              offset=t.offset + ch * chunk_stride
                              + a0 * row_stride,
                              ap=[t.ap[0], [row_stride, na], [0, 2],
                                  [1, WIN]])
                      drv = _pairs(dr.bitcast(f32), WIN, (XWR - 1) * WIN)
                      xwv = _pairs(xw.bitcast(f32), WIN, XWR * WIN)
                      xrc = xrk[:, ch, :, :]
                      nc.vector.tensor_tensor(out=xrc, in0=drv, in1=w1a_b,
                                              op=mybir.AluOpType.mult)
                      nc.vector.tensor_tensor(out=xrc, in0=xrc.bitcast(f32),
                                              in1=xwv, op=mybir.AluOpType.add)
                      nc.vector.tensor_tensor(out=xrc, in0=xrc.bitcast(f32),
                                              in1=upm_b,
                                              op=mybir.AluOpType.mult)
                  dck = dc[:, :, r0b:r1b, :]
                  nc.vector.tensor_tensor(out=dck,
                                          in0=xrk[:, :, :, 1:].bitcast(f32),
                                          in1=xrk[:, :, :, :-1].bitcast(f32),
                                          op=mybir.AluOpType.subtract)
                  nc.vector.tensor_copy(up[:, :, r0b:r1b, 1],
                                        xrk[:, :, :, 0].bitcast(f32))
                  nc.vector.tensor_copy(up[:, :, r0b:r1b, 1 + (W - 1)],
                                        xrk[:, :, :, WIN - 1].bitcast(f32))
                  dc_f = dck.bitcast(f32)[:, :, :, 0:WIN - 1]
                  xr_f = xrk.bitcast(f32)[:, :, :, 0:WIN - 1]
                  for (wt, col0) in ((w1bo, 2), (w1be, 3)):
                      out_v = bass.AP(tensor=up.tensor,
                                      offset=up.offset + r0b * WB + col0,
                                      ap=[up.ap[0], up.ap[1], [WB, nr],
                                          [2, WIN - 1]])
                      wt_b = bass.AP(tensor=wt.tensor, offset=wt.offset,
                                     ap=[wt.ap[0], [0, 2], [0, nr],
                                         wt.ap[1]])
                      nc.vector.tensor_tensor(out=out_v, in0=dc_f,
                                              in1=wt_b,
                                              op=mybir.AluOpType.mult)
                      nc.vector.tensor_tensor(out=out_v,
                                              in0=out_v.bitcast(f32),
                                              in1=xr_f,
                                              op=mybir.AluOpType.add)
                  # conv_up groups whose up-row window [u0, u0+nr+2) is now
                  # fully computed
                  while (cg_next < len(cg_list)
                         and cg_list[cg_next][0] + cg_list[cg_next][1] + 2
                         <= r1b):
                      emit_conv_up_group(*cg_list[cg_next])
                      cg_next += 1

              while cg_next < len(cg_list):
                  emit_conv_up_group(*cg_list[cg_next])
                  cg_next += 1

              upp_cm.__exit__(None, None, None)

              # mask out-of-image halo rows (data-driven per core); only the
              # first/last two rows can ever be masked, keep the ops tiny
              for rr in (0, Y0R - 2):
                  nc.vector.tensor_tensor(
                      out=y0[:, rr:rr + 2, 1:1 + W],
                      in0=y0[:, rr:rr + 2, 1:1 + W].bitcast(f32),
                      in1=bass.AP(tensor=my0.tensor, offset=my0.offset + rr,
                                  ap=[my0.ap[0], [1, 2], [0, W]]),
                      op=mybir.AluOpType.mult)

              # ---- conv r0 + bn1 + lrelu -> y1 ----
              y1 = sb.tile([C, Y1R, WB], f32r)
              nc.vector.memset(y1[:, :, 0:1].bitcast(f32), 0.0)
              nc.vector.memset(y1[:, :, WB - 1:WB].bitcast(f32), 0.0)
              for (v0, nr) in _row_groups(Y1R):
                  pt = psS.tile([C, nr * W], f32, tag="st")
                  for dy in range(3):
                      for dx in range(3):
                          k = 3 * dy + dx
                          nc.tensor.matmul(
                              pt, w0[:, k, :],
                              y0[:, v0 + dy:v0 + dy + nr, dx:dx + W],
                              start=(k == 0), stop=(k == 8))
                  nc.scalar.activation(out=y1[:, v0:v0 + nr, 1:1 + W],
                                       in_=pt.rearrange("p (r w) -> p r w", r=nr),
                                       func=mybir.ActivationFunctionType.Prelu,
                                       bias=b1, scale=1.0, alpha=alpha)
              for rr in (0, Y1R - 1):
                  nc.vector.tensor_tensor(
                      out=y1[:, rr:rr + 1, 1:1 + W],
                      in0=y1[:, rr:rr + 1, 1:1 + W].bitcast(f32),
                      in1=bass.AP(tensor=my1.tensor, offset=my1.offset + rr,
                                  ap=[my1.ap[0], [1, 1], [0, W]]),
                      op=mybir.AluOpType.mult)

              # ---- conv r1 + bn2 + lrelu -> y2 (flat [C, M]) ----
              y2 = sb.tile([C, HH, W], f32r)
              y2h = sb.tile([C, HH, W], bf16)
              for (z0, nr) in _row_groups(HH):
                  pt = psS.tile([C, nr * W], f32, tag="st")
                  for dy in range(3):
                      for dx in range(3):
                          k = 3 * dy + dx
                          nc.tensor.matmul(
                              pt, w1t[:, k, :],
                              y1[:, z0 + dy:z0 + dy + nr, dx:dx + W],
                              start=(k == 0), stop=(k == 8))
                  nc.scalar.activation(out=y2[:, z0:z0 + nr, :],
                                       in_=pt.rearrange("p (r w) -> p r w", r=nr),
                                       func=mybir.ActivationFunctionType.Prelu,
                                       bias=b2, scale=1.0, alpha=alpha)
                  nc.vector.tensor_copy(y2h[:, z0:z0 + nr, :],
                                        y2[:, z0:z0 + nr, :].bitcast(f32))
                  nc.sync.dma_start(
                      out=y2_dram[:, z0 * W:(z0 + nr) * W],
                      in_=y2h[:, z0:z0 + nr, :].rearrange("p r w -> p (r w)"))
              y2f = y2.rearrange("p r w -> p (r w)")

              # residual sum (attention adds xf + identity): y20 = y2 + y0_core
              # + bv (the attention v-bias collapses to a constant add since
              # softmax rows sum to 1), so the final lrelu is bias-free and
              # can run on the DVE instead of the busy scalar engine
              y20 = sb.tile([C, M], f32)
              nc.vector.tensor_tensor(
                  out=y20.rearrange("p (r w) -> p r w", r=HH),
                  in0=y2, in1=y0[:, 2:2 + HH, 1:1 + W].bitcast(f32),
                  op=mybir.AluOpType.add)
              bv_b = bass.AP(tensor=bv.tensor, offset=bv.offset,
                             ap=[bv.ap[0], [0, M]])
              nc.vector.tensor_tensor(out=y20, in0=y20, in1=bv_b,
                                      op=mybir.AluOpType.add)

              # ---- pair exchange: one AllReduce(add) per conv group pair;
              # the partner half is then (sum - own). Each core's attention
              # uses its own n-ordering [local | partner] (order-invariant).
              # Emitted first so the Pool queue dispatches it the moment the
              # last y2 DMA lands; all local-half attention below overlaps it.
              if os.environ.get("KERNEL_NO_COLLECTIVE", "0") == "1":
                  # timing probe only (wrong results): local copy in place of
                  # the pair exchange, to measure the collective's HW cost
                  nc.sync.dma_start(out=ar_out, in_=y2_dram)
              else:
                  nc.gpsimd.collective_compute(
                      "AllReduce", mybir.AluOpType.add,
                      replica_groups=GROUPS,
                      ins=[y2_dram.opt()],
                      outs=[ar_out.opt()])

              # ---- q (local only: overlaps the AllReduce) ----
              bq2_b = bass.AP(tensor=bq2.tensor, offset=bq2.offset,
                              ap=[bq2.ap[0], [0, 512]])
              # q2: [wq|wq] stationary writes q into partitions 0..63 AND a
              # copy into 64..127 in one matmul, so S^T n-chunk pairs can run
              # as concurrent row-group-packed matmuls.
              q2 = sb.tile([128, M], f32r)
              for c0 in range(0, M, 512):
                  pq = psS.tile([128, 512], f32, tag="st")
                  nc.tensor.matmul(pq, wq2, y2f[:, c0:c0 + 512], start=True, stop=True)
                  nc.vector.tensor_tensor(out=q2[:, c0:c0 + 512], in0=pq,
                                          in1=bq2_b, op=mybir.AluOpType.add)

              # ---- k, vT ----
              bk2_b = bass.AP(tensor=bk2.tensor, offset=bk2.offset,
                              ap=[bk2.ap[0], [0, 512]])
              # local and remote halves live in SEPARATE tiles and are
              # emitted in a schedule that keeps ALL local-half attention
              # (S^T/exp/O/sum for every query quarter, with PSUM partials
              # spilled to SBUF) queued ahead of the first collective-
              # dependent instruction: engines drain queues in order, so
              # anything emitted after a stalled instruction waits with it.
              MJ = M // 128
              k2h = [sb.tile([128, M], f32r, name=f"k2h{h}")
                     for h in range(2)]
              vTh = [sb.tile([128, MJ, C], f32r, name=f"vTh{h}")
                     for h in range(2)]

              def project_half(half):
                  src_half = y2f if half == 0 else xrem
                  for c0 in range(0, M, 512):
                      pk = psS.tile([128, 512], f32, tag="st")
                      nc.tensor.matmul(pk, wk2, src_half[:, c0:c0 + 512],
                                       start=True, stop=True)
                      nc.vector.tensor_tensor(out=k2h[half][:, c0:c0 + 512],
                                              in0=pk, in1=bk2_b,
                                              op=mybir.AluOpType.add)
                  for jj in range(MJ):
                      pv = psS.tile([128, 2, C], f32, tag="st")
                      nc.tensor.matmul(pv, src_half[:, jj * 128:(jj + 1) * 128],
                                       wvt2.rearrange("p a b -> p (a b)"),
                                       start=True, stop=True)
                      if half == 0 and jj % 2 == 1:
                          nc.scalar.activation(
                              out=vTh[half][:, jj, :], in_=pv[:, 0, :],
                              func=mybir.ActivationFunctionType.Copy)
                      else:
                          nc.vector.tensor_copy(vTh[half][:, jj, :],
                                                pv[:, 0, :])

              NJ = N // 128
              NJ2 = NJ // 2

              def attn_quarter(ms, pO, psums, half):
                  mlo = ms * 512
                  j_first = half * NJ2
                  j_last = j_first + NJ2 - 1
                  for j0 in range(j_first, j_first + NJ2, 2):
                      pS = [psS.tile([128, 512], f32, tag="st",
                                     name=f"pS{u}")
                            for u in range(2)]
                      for u in range(2):
                          j = j0 + u
                          bp0 = u * D
                          nc.tensor.matmul(pS[u],
                                           k2h[j // MJ][bp0:bp0 + D,
                                                        (j % MJ) * 128:
                                                        (j % MJ + 1) * 128],
                                           q2[bp0:bp0 + D, mlo:mlo + 512],
                                           start=True, stop=True,
                                           tile_position=(bp0, 0))
                      for u in range(2):
                          j = j0 + u
                          pt = pts.tile([128, 512], f32r, tag="pt")
                          nc.scalar.activation(
                              out=pt, in_=pS[u],
                              func=mybir.ActivationFunctionType.Exp,
                              bias=nshift, scale=1.0)
                          nc.tensor.matmul(pO, vTh[j // MJ][:, j % MJ, :],
                                           pt,
                                           start=(j == j_first),
                                           stop=(j == j_last))
                          nc.tensor.matmul(psums, onesc, pt,
                                           start=(j == j_first),
                                           stop=(j == j_last))

              # SBUF spill slots for the local-half partial O and row sums:
              # frees all PSUM while the collective is in flight so every
              # quarter's local half can queue ahead of it
              oloc = sb.tile([C, 4, 512], f32)
              sloc = sb.tile([1, 4, 512], f32)

              def attn_finish(ms, pO, psums):
                  mlo = ms * 512
                  sums = pts.tile([1, 512], f32, tag="sm")
                  nc.vector.tensor_tensor(out=sums, in0=psums,
                                          in1=sloc[:, ms, :],
                                          op=mybir.AluOpType.add)
                  recip = pts.tile([1, 512], f32, tag="rc")
                  with nc.allow_low_precision(reason="softmax denominator"):
                      nc.vector.reciprocal(out=recip, in_=sums)
                  rbs = pts.tile([128, 512], f32, tag="rb")
                  nc.gpsimd.partition_broadcast(rbs, recip)
                  onorm = pts.tile([C, 512], f32, tag="on")
                  nc.vector.tensor_tensor(out=onorm, in0=pO,
                                          in1=oloc[:, ms, :],
                                          op=mybir.AluOpType.add)
                  nc.vector.tensor_tensor(out=onorm, in0=onorm, in1=rbs,
                                          op=mybir.AluOpType.mult)
                  nc.vector.tensor_tensor(out=onorm, in0=onorm,
                                          in1=y20[:, mlo:mlo + 512],
                                          op=mybir.AluOpType.add)
                  osb = pts.tile([C, 512], f32, tag="ob")
                  nc.vector.tensor_scalar_mul(out=osb, in0=onorm, scalar1=ALPHA)
                  nc.vector.tensor_tensor(out=osb, in0=osb, in1=onorm,
                                          op=mybir.AluOpType.max)
                  nc.sync.dma_start(out=out_ap[:, ms * 8:(ms + 1) * 8, :],
                                    in_=osb.rearrange("p (r w) -> p r w", r=8))

              project_half(0)
              # ALL local-half attention quarters queue ahead of anything
              # that needs the collective; PSUM partials spill to SBUF
              for ms in range(4):
                  pO = psO.tile([C, 512], f32, tag="po")
                  psums = psA.tile([1, 512], f32, tag="psum")
                  attn_quarter(ms, pO, psums, 0)
                  nc.scalar.activation(
                      out=oloc[:, ms, :], in_=pO,
                      func=mybir.ActivationFunctionType.Copy)
                  nc.vector.tensor_copy(sloc[:, ms, :], psums)
              # partner half from the AllReduce: xrem = (bf16 sum) - own.
              # First collective-dependent instructions on the DMA/DVE queues.
              xremh = sb.tile([C, M], bf16)
              xrem = sb.tile([C, M], f32r)
              for c0 in range(0, M, 512):
                  nc.sync.dma_start(out=xremh[:, c0:c0 + 512],
                                    in_=ar_out[:, c0:c0 + 512])
                  nc.vector.tensor_tensor(
                      out=xrem[:, c0:c0 + 512],
                      in0=xremh[:, c0:c0 + 512],
                      in1=y2f[:, c0:c0 + 512].bitcast(f32),
                      op=mybir.AluOpType.subtract)
              # remote-half projections (first collective-dependent PE work)
              project_half(1)
              for ms in range(4):
                  pO = psO.tile([C, 512], f32, tag="po")
                  psums = psA.tile([1, 512], f32, tag="psum")
                  attn_quarter(ms, pO, psums, 1)
                  attn_finish(ms, pO, psums)

    nc.compile()
    if reps == 1:
        _PROGRAM = nc
    return nc


def _prep_inputs(x, W_up, b_up, g0, be0, m0, v0, W_r0, g1, be1, m1, v1,
                 W_r1, g2, be2, m2, v2, Wq, bq, Wk, bk, Wv, bv):
    """Build the 8 per-core input maps (host-side sharding/packing only)."""
    x = np.asarray(x, np.float32)

    def fold(wc, scale):
        return (wc * scale[:, None, None, None]).astype(np.float32)

    def pack(wc):  # [co, ci, 3, 3] -> [ci, 9, co]
        return np.ascontiguousarray(
            wc.transpose(1, 2, 3, 0).reshape(wc.shape[1], 9, wc.shape[0]))

    s0 = np.asarray(g0) / np.sqrt(np.asarray(v0) + EPS)
    s1 = np.asarray(g1) / np.sqrt(np.asarray(v1) + EPS)
    s2 = np.asarray(g2) / np.sqrt(np.asarray(v2) + EPS)
    b0f = (np.asarray(b_up) * s0 + np.asarray(be0) - np.asarray(m0) * s0)
    b1f = (np.asarray(be1) - np.asarray(m1) * s1)
    b2f = (np.asarray(be2) - np.asarray(m2) * s2)

    wu_p = pack(fold(np.asarray(W_up), s0))      # [256, 9, 128]
    wu_p = wu_p.reshape(2, 128, 9, C).transpose(1, 0, 2, 3)
    wu_p = np.ascontiguousarray(wu_p, np.float32)
    w0_p = np.ascontiguousarray(pack(fold(np.asarray(W_r0), s1)), np.float32)
    w1_p = np.ascontiguousarray(pack(fold(np.asarray(W_r1), s2)), np.float32)

    co = np.linspace(0.0, HIN - 1.0, H)
    i0 = np.floor(co).astype(np.int64)
    i1 = np.minimum(i0 + 1, HIN - 1)
    wrow = (co - i0).astype(np.float32)
    w1b_col = (co - i0).astype(np.float32)       # same grid for W axis
    w1bo_t = np.broadcast_to(w1b_col[1:63:2][None, :], (128, WIN - 1)).copy()
    w1be_t = np.broadcast_to(w1b_col[2:63:2][None, :], (128, WIN - 1)).copy()

    wqt = np.ascontiguousarray(np.asarray(Wq).T, np.float32)
    wkt = np.ascontiguousarray(np.asarray(Wk).T, np.float32)
    wq2 = np.ascontiguousarray(np.concatenate([wqt, wqt], axis=1), np.float32)
    wk2 = np.ascontiguousarray(np.concatenate([wkt, wkt], axis=1), np.float32)
    wvt = np.ascontiguousarray(np.asarray(Wv).T, np.float32)
    bq2_c = np.concatenate([np.asarray(bq), np.asarray(bq)]).astype(
        np.float32).reshape(2 * D, 1)
    bk2_c = np.concatenate([np.asarray(bk), np.asarray(bk)]).astype(
        np.float32).reshape(2 * D, 1)
    bv_c = np.asarray(bv, np.float32).reshape(C, 1)
    b0c = b0f.astype(np.float32).reshape(C, 1)
    b1c = b1f.astype(np.float32).reshape(C, 1)
    b2c = b2f.astype(np.float32).reshape(C, 1)

    XWR = 22
    in_maps = []
    for core in range(N_CORES):
        s, h = core // 2, core % 2
        base = HH * h
        xs = x[s]                                # [256, 32, 32]
        rlo = base // 2 - 2
        xw = np.zeros((CIN, XWR, WIN), np.float32)
        for r in range(XWR):
            xr_idx = rlo + r
            if 0 <= xr_idx < HIN:
                xw[:, r, :] = xs[:, xr_idx, :]
        xw = np.ascontiguousarray(
            xw.reshape(2, 128, XWR, WIN).transpose(1, 0, 2, 3))
        w1a = np.zeros((UPR,), np.float32)
        upm = np.zeros((UPR,), np.float32)
        for t in range(UPR):
            j = base - 3 + t
            if 0 <= j < H:
                upm[t] = 1.0
                # j==0 is the exact-sample row: through the pair formula
                # x[rel a] + w*(x[rel a+1]-x[rel a]) with w=1 it returns
                # x[rel a+1] = x row 0 exactly
                w1a[t] = 1.0 if j == 0 else wrow[j]
        w1a_t = np.broadcast_to(w1a[None, :], (128, UPR)).copy()
        upm_t = np.broadcast_to(upm[None, :], (128, UPR)).copy()

        my0 = np.ones((Y0R,), np.float32)
        my1 = np.ones((Y1R,), np.float32)
        if h == 0:
            my0[0:2] = 0.0                       # y0 rows -2,-1
            my1[0] = 0.0                         # y1 row -1
        else:
            my0[Y0R - 2:] = 0.0                  # y0 rows 64,65
            my1[Y1R - 1] = 0.0                   # y1 row 64
        in_maps.append(dict(
            xw=xw, upm=upm_t,
            w1a=w1a_t, w1bo=w1bo_t, w1be=w1be_t,
            wu=wu_p, w0=w0_p, w1=w1_p,
            b0=b0c, b1=b1c, b2=b2c,
            my0=np.broadcast_to(my0[None, :], (128, Y0R)).copy(),
            my1=np.broadcast_to(my1[None, :], (128, Y1R)).copy(),
            wq2=wq2, bq2=bq2_c, wk2=wk2, bk2=bk2_c, wvt=wvt, bv=bv_c,
            onesc=np.ones((128, 1), np.float32),
        ))
    return in_maps


def kernel(**inputs):
    global LAST_RUN
    nc = build_program()
    in_maps = _prep_inputs(**inputs)
    trace = bool(int(os.environ.get("KERNEL_TRACE", "0")))
    res = run_bass_kernel_spmd(nc, in_maps, list(range(N_CORES)), trace=trace)
    LAST_RUN = res
    out = np.empty((B, C, H, W), np.float32)
    for core in range(N_CORES):
        s, h = core // 2, core % 2
        out[s, :, HH * h:HH * (h + 1), :] = res.results[core]["out"]
    return out

